# revision 1
# baseline (speedup 1.0000x reference)
import sys
sys.path.insert(0, "/opt/trn_rl_repo")
import numpy as np

WS, NH, C, HS = 7, 8, 128, 16
N = WS * WS
M113 = 113
KBW = 128
B, H, W = 16, 112, 112
NCORES = 8
B_PER_CORE = B // NCORES
NBANDS = H // WS
TOK_BAND = WS * W
NWIN = W // WS
TOK_CORE = B_PER_CORE * H * W

_COLS = dict(wq=C, wkA=C, wkB=C, wv=C, wo=C, eb=4 * N, i112=112, i49=N, bo=C,
             o112=C)
CB_TOTAL = sum(_COLS.values())

_CACHE = {}


def _build_module(reps=1):
    import concourse.bass as bass
    import concourse.mybir as mybir
    import concourse.tile as tile
    from concourse import bacc
    from contextlib import ExitStack

    F32 = mybir.dt.float32
    BF16 = mybir.dt.bfloat16
    nc = bacc.Bacc(None)
    xin = nc.declare_dram_parameter("xin", [TOK_CORE, C], F32, isOutput=False)
    cblob = nc.declare_dram_parameter("cblob", [C, CB_TOTAL], BF16,
                                      isOutput=False)
    cfp = nc.declare_dram_parameter("cfp", [112, 112], F32, isOutput=False)
    out = nc.declare_dram_parameter("out", [TOK_CORE, C], F32, isOutput=True)

    with tile.TileContext(nc) as tc, ExitStack() as ctx:
        singles = ctx.enter_context(tc.tile_pool(name="singles", bufs=1))
        sb = ctx.enter_context(tc.tile_pool(name="sb", bufs=2))
        ps = ctx.enter_context(tc.tile_pool(name="ps", bufs=4, space="PSUM"))

        cb = singles.tile([C, CB_TOTAL], BF16, tag="cblob", name="cblob_t")
        nc.sync.dma_start(cb[:], cblob[:])
        i112f = singles.tile([112, 112], F32, tag="cfp", name="cfp_t")
        nc.sync.dma_start(i112f[:], cfp[:])
        ofs = {}
        o = 0
        for k, w_ in _COLS.items():
            ofs[k] = o
            o += w_

        def cs(key, p0=0, p1=C, c0=0, c1=None):
            c1 = _COLS[key] if c1 is None else c1
            return cb[p0:p1, ofs[key] + c0:ofs[key] + c1]

        dummy_ps = ps.tile([C, 1], F32, tag="pp", name="dummy_ps")
        nc.tensor.matmul(dummy_ps[:, 0:1], lhsT=cs("wq"), rhs=cs("wq", c1=1),
                         start=True, stop=True)

        NKV = 3
        kblk_bufs, vplus_bufs = [], []
        for pb in range(NKV):
            kb = sb.tile([C, NWIN * KBW], BF16, tag="kblk", name=f"kblk_b{pb}", bufs=NKV)
            nc.vector.memset(kb[:], 0.0)
            kblk_bufs.append(kb)
            vp = sb.tile([C, NWIN * 136], BF16, tag="vplus",
                         name=f"vplus_b{pb}", bufs=NKV)
            nc.gpsimd.memset(vp[:], 0.0)
            va = vp[:]
            ones_top = bass.AP(tensor=va.tensor, offset=va.offset + 32,
                               ap=[[va.ap[0][0], N], [136, NWIN], [34, 4]])
            nc.gpsimd.memset(ones_top, 1.0)
            ones_bot = bass.AP(tensor=va.tensor,
                               offset=va.offset + 64 * va.ap[0][0] + 33,
                               ap=[[va.ap[0][0], N], [136, NWIN], [34, 4]])
            nc.gpsimd.memset(ones_bot, 1.0)
            vplus_bufs.append(vp)
        ot_bufs = []
        for pb in range(2):
            otb = sb.tile([C, WS * 128], BF16, tag="ot", name=f"ot_b{pb}")
            ota0 = otb[:]
            pad = bass.AP(tensor=ota0.tensor, offset=ota0.offset + 112,
                          ap=[[ota0.ap[0][0], C], [128, WS], [1, 16]])
            nc.vector.memset(pad, 0.0)
            ot_bufs.append(otb)

        rep_cm = tc.For_i(0, reps, 1) if reps > 1 else None
        if rep_cm is not None:
            rep_cm.__enter__()

        _state = {}

        def emit_head(chunk):
            img, band = divmod(chunk, NBANDS)
            base = img * H * W + band * TOK_BAND
            cn = f"c{chunk}"

            xnat = sb.tile([112, WS * C], F32, tag="xnat", name=f"xnat_{cn}")
            xi = xin[:]
            src = bass.AP(tensor=xi.tensor, offset=xi.offset + base * C,
                          ap=[[C, 112], [112 * C, WS], [1, C]])
            nc.sync.dma_start(xnat[:], src)

            xt_p = [ps.tile([C, 448], F32, tag="pp", name=f"xtp{t}_{cn}")
                    for t in range(2)]
            for r in range(WS):
                t_, rl = (0, r) if r < 4 else (1, r - 4)
                nc.tensor.transpose(xt_p[t_][:, 112 * rl:112 * (rl + 1)],
                                    xnat[0:112, C * r:C * (r + 1)],
                                    i112f[:])
            xt_sb = sb.tile([C, TOK_BAND], BF16, tag="xt", name=f"xt_{cn}")
            xta = xt_sb[:]
            for t_, nr, r0 in ((0, 4, 0), (1, 3, 4)):
                dst = bass.AP(tensor=xta.tensor, offset=xta.offset + 7 * r0,
                              ap=[list(xta.ap[0]), [7, nr], [49, 16], [1, 7]])
                sv = xt_p[t_].rearrange("p (r ws) -> p r ws", r=4)[:, 0:nr, :]
                sv4 = sv.rearrange("p r (w s) -> p r w s", w=16)
                if t_ == 0:
                    nc.scalar.activation(dst, sv4,
                                         mybir.ActivationFunctionType.Copy)
                else:
                    nc.vector.tensor_copy(dst, sv4)

            qt_sb = sb.tile([C, TOK_BAND], BF16, tag="qt", name=f"qt_{cn}")
            kblk = kblk_bufs[chunk % NKV]
            ka = kblk[:]
            for half in range(2):
                qp = ps.tile([C, 392], F32, tag="pp", name=f"qp{half}_{cn}")
                nc.tensor.matmul(qp[:], lhsT=cs("wq"),
                                 rhs=xt_sb[:, 392 * half:392 * (half + 1)],
                                 start=True, stop=True)
                nc.scalar.activation(qt_sb[:, 392 * half:392 * (half + 1)],
                                     qp[:],
                                     mybir.ActivationFunctionType.Copy)
                for key, c0 in (("wkA", 0), ("wkB", 64)):
                    kp = ps.tile([C, 392], F32, tag="pp", name=f"kp{key}{half}_{cn}")
                    nc.tensor.matmul(kp[:], lhsT=cs(key),
                                     rhs=xt_sb[:, 392 * half:392 * (half + 1)],
                                     start=True, stop=True)
                    dst = bass.AP(tensor=ka.tensor,
                                  offset=ka.offset + (8 * half) * KBW + c0,
                                  ap=[list(ka.ap[0]), [KBW, 8], [1, N]])
                    srcv = kp.rearrange("p (w n) -> p w n", w=8)
                    nc.vector.tensor_copy(dst, srcv)

            vplus = vplus_bufs[chunk % NKV]
            vv = vplus.rearrange("p (w g s) -> p w g s", w=NWIN, g=4)
            for vq in range(2):
                vp = ps.tile([C, 512], F32, tag="pp", name=f"vp{vq}_{cn}")
                for wl in range(8):
                    w_ = 8 * vq + wl
                    xg = xt_sb[:, N * w_:N * (w_ + 1)]
                    nc.tensor.matmul(vp[0:N, 64 * wl:64 * wl + 64],
                                     lhsT=xg, rhs=cs("wv", c0=0, c1=64),
                                     start=True, stop=True, tile_position=(0, 0))
                    nc.tensor.matmul(vp[64:64 + N, 64 * wl:64 * wl + 64],
                                     lhsT=xg, rhs=cs("wv", c0=64, c1=128),
                                     start=True, stop=True, tile_position=(0, 64))
                vpv = vp.rearrange("p (w g s) -> p w g s", w=8, g=4)
                nc.scalar.activation(vv[0:N, 8 * vq:8 * vq + 8, :, 0:HS],
                                     vpv[0:N, :, :, :],
                                     mybir.ActivationFunctionType.Copy)
                nc.scalar.activation(vv[64:64 + N, 8 * vq:8 * vq + 8, :, HS:2 * HS],
                                     vpv[64:64 + N, :, :, :],
                                     mybir.ActivationFunctionType.Copy)

            p_sb = sb.tile([C, NWIN * 4 * N], BF16, tag="psb", name=f"psb_{cn}")
            pa = p_sb[:]
            for half in range(2):
                s_ps = ps.tile([C, 2048], F32, tag="sps", name=f"sps{half}_{cn}",
                               bufs=1)
                for wl in range(8):
                    w_ = 8 * half + wl
                    for g in range(4):
                        nc.tensor.matmul(
                            s_ps[0:KBW, 512 * g + N * wl:512 * g + N * (wl + 1)],
                            lhsT=kblk[32 * g:32 * (g + 1),
                                      KBW * w_:KBW * (w_ + 1)],
                            rhs=qt_sb[32 * g:32 * (g + 1), N * w_:N * (w_ + 1)],
                            start=True, stop=True, tile_position=(32 * g, 0))
                sa = s_ps[:]
                src_ap = bass.AP(tensor=sa.tensor, offset=sa.offset,
                                 ap=[[sa.ap[0][0], M113], [512, 4], [N, 8], [1, N]])
                dst_ap = bass.AP(tensor=pa.tensor,
                                 offset=pa.offset + (4 * (8 * half)) * N,
                                 ap=[[pa.ap[0][0], M113], [N, 4], [4 * N, 8], [1, N]])
                nc.scalar.activation(dst_ap, src_ap,
                                     mybir.ActivationFunctionType.Exp)
                for eng, w0, nw in ((nc.gpsimd, 0, 8),):
                    pdst = bass.AP(
                        tensor=pa.tensor,
                        offset=pa.offset + (4 * (8 * half + w0)) * N,
                        ap=[[pa.ap[0][0], M113], [4 * N, nw], [1, 4 * N]])
                    ebsrc = bass.AP(
                        tensor=cb[:].tensor,
                        offset=cb[:].offset + ofs["eb"],
                        ap=[[cb[:].ap[0][0], M113], [0, nw], [1, 4 * N]])
                    eng.tensor_tensor(pdst, pdst, ebsrc,
                                      op=mybir.AluOpType.mult)

            _state[chunk] = (base, cn, xt_sb, qt_sb, p_sb, vplus)

        def emit_tail(chunk):
            base, cn, xt_sb, qt_sb, p_sb, vplus = _state.pop(chunk)
            pa = p_sb[:]
            onorm = sb.tile([C, 8 * C], BF16, tag="onorm", name=f"onorm_{cn}")
            ona = onorm[:]
            pg_sizes = ((0, 3), (3, 6), (6, 8))
            for pg0, pg1 in pg_sizes:
                npair = pg1 - pg0
                o_ps = ps.tile([C, 136 * npair], F32, tag="pp",
                               name=f"ops{pg0}_{cn}")
                for pl in range(npair):
                    for g in range(4):
                        for wl in range(2):
                            w_ = 2 * (pg0 + pl) + wl
                            b0 = 64 * wl
                            scol = (4 * w_ + g) * N
                            nc.tensor.matmul(
                                o_ps[b0:b0 + N, 136 * pl + 34 * g:
                                     136 * pl + 34 * (g + 1)],
                                lhsT=p_sb[0:M113, scol:scol + N],
                                rhs=vplus[0:M113, 136 * w_ + 34 * g:
                                          136 * w_ + 34 * (g + 1)],
                                start=True, stop=True, tile_position=(0, b0))
                recip = sb.tile([C, 8 * 3], F32, tag="recip",
                                name=f"rc{pg0}_{cn}")
                oa = o_ps[:]
                ra = recip[:]
                pp_o = oa.ap[0][0]
                pp_r = ra.ap[0][0]
                for wl in range(2):
                    b0 = 64 * wl
                    den = bass.AP(tensor=oa.tensor,
                                  offset=oa.offset + b0 * pp_o + 32,
                                  ap=[[pp_o, N], [34, 4 * npair], [1, 2]])
                    rc = bass.AP(tensor=ra.tensor, offset=ra.offset + b0 * pp_r,
                                 ap=[[pp_r, N], [1, 8 * npair]])
                    nc.vector.reciprocal(rc, den)
                    for hh in range(2):
                        src_o = bass.AP(tensor=oa.tensor,
                                        offset=oa.offset + b0 * pp_o + 16 * hh,
                                        ap=[[pp_o, N], [136, npair], [34, 4],
                                            [1, HS]])
                        rb = bass.AP(tensor=ra.tensor,
                                     offset=ra.offset + b0 * pp_r + hh,
                                     ap=[[pp_r, N], [8, npair], [2, 4],
                                         [0, HS]])
                        dst_o = bass.AP(tensor=ona.tensor,
                                        offset=ona.offset + b0 * ona.ap[0][0]
                                        + C * pg0 + 16 * hh,
                                        ap=[[ona.ap[0][0], N], [C, npair],
                                            [32, 4], [1, HS]])
                        nc.vector.tensor_tensor(dst_o, src_o, rb,
                                                op=mybir.AluOpType.mult)

            ot_sb = ot_bufs[chunk % 2]
            ota = ot_sb[:]
            otE = ps.tile([C, 392], F32, tag="pp", name=f"otE_{cn}")
            otO = ps.tile([C, 392], F32, tag="pp", name=f"otO_{cn}")
            for w_ in range(NWIN):
                wl, pair = w_ % 2, w_ // 2
                b0 = 64 * wl
                opst = otE if wl == 0 else otO
                nc.tensor.matmul(opst[:, N * pair:N * (pair + 1)],
                                 lhsT=onorm[b0:b0 + N, C * pair:C * (pair + 1)],
                                 rhs=cs("i49", b0, b0 + N),
                                 start=True, stop=True, tile_position=(b0, 0))
            for wl, src_t in ((0, otE), (1, otO)):
                dst = bass.AP(tensor=ota.tensor, offset=ota.offset + 7 * wl,
                              ap=[list(ota.ap[0]), [14, 8], [128, 7], [1, 7]])
                sv = src_t.rearrange("p (w r s) -> p w r s", w=8, r=7)
                if wl == 0:
                    nc.scalar.activation(dst, sv,
                                         mybir.ActivationFunctionType.Copy)
                else:
                    nc.vector.tensor_copy(dst, sv)

            f_ps = [ps.tile([C, 512], F32, tag="pp", name=f"fp{t}_{cn}")
                    for t in range(2)]
            ba = cb[:]
            for t_, nr in ((0, 4), (1, 3)):
                bo_r = bass.AP(tensor=ba.tensor, offset=ba.offset + ofs["bo"],
                               ap=[[ba.ap[0][0], 1], [0, nr], [1, C]])
                nc.tensor.matmul(f_ps[t_][0:C, 0:128 * nr],
                                 lhsT=cs("o112", 0, 1), rhs=bo_r,
                                 start=True, stop=False, skip_group_check=True)
            for r in range(WS):
                t_, rl = (0, r) if r < 4 else (1, r - 4)
                last = (r == 3) or (r == 6)
                nc.tensor.matmul(f_ps[t_][0:C, 128 * rl:128 * (rl + 1)],
                                 lhsT=ot_sb[:, 128 * r:128 * (r + 1)],
                                 rhs=cs("wo"),
                                 start=False, stop=last,
                                 skip_group_check=True, tile_position=(0, 0))
            fin = sb.tile([C, 896], F32, tag="fin", name=f"fin_{cn}")
            for t_, nr in ((0, 4), (1, 3)):
                fdst = fin[0:112, 512 * t_:512 * t_ + 128 * nr]
                fsrc = f_ps[t_][0:112, 0:128 * nr]
                if t_ == 0:
                    nc.scalar.activation(fdst, fsrc,
                                         mybir.ActivationFunctionType.Copy)
                else:
                    nc.vector.tensor_copy(fdst, fsrc)
            oap = out[:]
            fa = fin[:]
            dst = bass.AP(tensor=oap.tensor, offset=oap.offset + base * C,
                          ap=[[C, 112], [112 * C, WS], [1, C]])
            s2 = bass.AP(tensor=fa.tensor, offset=fa.offset,
                         ap=[[fa.ap[0][0], 112], [C, WS], [1, C]])
            nc.sync.dma_start(dst, s2)

        NCH = B_PER_CORE * NBANDS
        emit_head(0)
        for chunk in range(NCH):
            if chunk + 1 < NCH:
                emit_head(chunk + 1)
            emit_tail(chunk)

        if rep_cm is not None:
            rep_cm.__exit__(None, None, None)

    nc.finalize()
    return nc


def _head_perm():
    perm = np.zeros(C, np.int64)
    for g in range(4):
        perm[32 * g:32 * g + 16] = np.arange(16) + 16 * g
        perm[32 * g + 16:32 * g + 32] = np.arange(16) + 16 * (g + 4)
    return perm


def _rel_index():
    coords = np.stack(np.meshgrid(np.arange(WS), np.arange(WS),
                                  indexing="ij"), 0).reshape(2, -1)
    rel = coords[:, :, None] - coords[:, None, :] + (WS - 1)
    return rel[0] * (2 * WS - 1) + rel[1]


def _build_cblob(w_q, w_k, w_v, w_o, b_o, rel_bias):
    import ml_dtypes
    BF = ml_dtypes.bfloat16
    scale = HS ** -0.5
    perm = _head_perm()
    wq_dev = (w_q * scale)[:, perm].astype(np.float32)
    wk_dev = w_k[:, perm].astype(np.float32)
    wkA = wk_dev.copy()
    wkB = wk_dev.copy()
    for g in range(4):
        wkA[:, 32 * g + 16:32 * g + 32] = 0.0
        wkB[:, 32 * g:32 * g + 16] = 0.0

    bias = rel_bias[_rel_index()].transpose(2, 0, 1).astype(np.float32)
    eb = np.ones((C, 4 * N), np.float32)
    for g in range(4):
        eb[0:N, N * g:N * (g + 1)] = np.exp(bias[g]).T
        eb[64:64 + N, N * g:N * (g + 1)] = np.exp(bias[g + 4]).T

    i112 = np.zeros((C, 112), np.float32)
    i112[0:112, :] = np.eye(112)
    i49 = np.zeros((C, N), np.float32)
    for b0 in (0, 64):
        i49[b0:b0 + N, :] = np.eye(N)
    bo_rep = np.broadcast_to(np.asarray(b_o, np.float32), (C, C))

    perm2 = np.zeros(C, np.int64)
    for g in range(4):
        for hh in range(2):
            perm2[32 * g + 16 * hh:32 * g + 16 * hh + 16] = \
                np.arange(16) + 16 * (g + 4 * hh)
    wo_dev = np.asarray(w_o, np.float32)[perm2, :]
    parts = dict(wq=wq_dev, wkA=wkA, wkB=wkB, wv=np.asarray(w_v, np.float32),
                 wo=wo_dev, eb=eb, i112=i112, i49=i49, bo=bo_rep,
                 o112=np.ones((C, C), np.float32))
    blob = np.concatenate([np.ascontiguousarray(parts[k], dtype=np.float32)
                           for k in _COLS], axis=1)
    assert blob.shape == (C, CB_TOTAL)
    return np.ascontiguousarray(blob.astype(BF))


def kernel(x, w_q, w_k, w_v, w_o, b_o, rel_bias):
    from concourse.bass_utils import run_bass_kernel_spmd

    import os
    x = np.asarray(x, np.float32)
    reps = int(os.environ.get("BLOCKSA_REPS", "1"))
    key = f"nc{reps}"
    if key not in _CACHE:
        _CACHE[key] = _build_module(reps)
    nc = _CACHE[key]

    cblob = _build_cblob(np.asarray(w_q, np.float32), np.asarray(w_k, np.float32),
                         np.asarray(w_v, np.float32), np.asarray(w_o, np.float32),
                         np.asarray(b_o, np.float32),
                         np.asarray(rel_bias, np.float32))

    cfp = np.ascontiguousarray(np.eye(112, dtype=np.float32))
    in_maps = []
    for c in range(NCORES):
        shard = x[B_PER_CORE * c:B_PER_CORE * (c + 1)].reshape(TOK_CORE, C)
        in_maps.append(dict(xin=np.ascontiguousarray(shard), cblob=cblob,
                            cfp=cfp))

    res = run_bass_kernel_spmd(nc, in_maps, list(range(NCORES)))
    outs = [res.results[c]["out"].reshape(B_PER_CORE, H, W, C)
            for c in range(NCORES)]
    return np.concatenate(outs, axis=0)

